# revision 1
# baseline (speedup 1.0000x reference)
"""AudioEncoder Trainium2 kernel.

Computes: conv1d(1->64, k=5, stride=2, pad=2) + bias -> ReLU -> per-timestep
linear (64->64) + bias, over audio [4, 480000] f32 -> out [4, 240000, 64] f32.

Strategy (pure data parallel over 8 cores):
  - Each core handles one half of one batch row: S = 120000 output positions.
  - Host pre-pads/casts audio to fp16 and de-interleaves it into even/odd
    streams xe[i] = xp[2i], xo[i] = xp[2i+1] (xp[t] = x[t-2] zero-padded), so
    the on-chip im2col rows are contiguous DMA reads:
      row0 = xe[j]   (tap 0)   row3 = xo[j]   (tap 1)
      row1 = xe[1+j] (tap 2)   row4 = xo[1+j] (tap 3)
      row2 = xe[2+j] (tap 4)
    (conv weights are host-reordered to [k=0,2,4,1,3] to match).
  - Conv: K=5 fp16 matmul; the moving operand uses a permuted 3D AP so that
    within each 512-position half, PSUM column c = t*128 + r holds position
    j0 + 4r + t.  Two col-group-packed matmuls fill PSUM [128, 512] with two
    halves (partitions 0-63 / 64-127).
  - ACT applies conv bias + ReLU, evacuating PSUM -> SBUF fp16 feats.
  - Linear: K=64 fp16 matmuls, feats tiles stationary, lin_w.T moving.  The
    A (feats rows 0-63) and B (rows 64-127) matmuls write SEPARATE PSUM banks:
    row-group-tiled matmuls writing the same partitions of one bank
    concurrently is a hardware fault (per-partition PSUM write-port conflict).
  - DVE adds the (pre-broadcast) linear bias while evacuating to SBUF.
  - Thanks to the position permutation each SBUF partition holds 4+4
    consecutive output rows, so the store DMA moves 1 KiB-contiguous runs.
"""

import numpy as np

import concourse.bacc as bacc
import concourse.bass as bass
import concourse.mybir as mybir
import concourse.tile as tile
from concourse.bass_utils import run_bass_kernel_spmd

B = 4
T = 480000
S_FULL = 240000  # conv output positions per batch row
N_CORES = 8
S_CORE = S_FULL * B // N_CORES  # 120000 positions per core
CHUNK = 1024  # output positions per inner chunk (two 512 halves)
SUPER = 8192  # output positions covered per im2col load
E = 64  # conv out channels
P = 64  # linear out features
KS = 5

f16 = mybir.dt.float16
f32 = mybir.dt.float32


def emit(nc: bass.Bass, S: int = S_CORE) -> None:
    """Emit the per-core Tile kernel for S output positions."""
    from contextlib import ExitStack

    xe_d = nc.declare_dram_parameter("xe", [S + 2], f16, isOutput=False)
    xo_d = nc.declare_dram_parameter("xo", [S + 2], f16, isOutput=False)
    wc_d = nc.declare_dram_parameter("wc", [KS, E], f16, isOutput=False)
    cb_d = nc.declare_dram_parameter("cb", [128, 1], f32, isOutput=False)
    w2_d = nc.declare_dram_parameter("w2", [128, P], f16, isOutput=False)
    b2_d = nc.declare_dram_parameter("b2", [128, 8 * P], f32, isOutput=False)
    out_d = nc.declare_dram_parameter("out", [S, P], f32, isOutput=True)

    RELU = mybir.ActivationFunctionType.Relu
    HALF = CHUNK // 2

    with tile.TileContext(nc) as tc, ExitStack() as ctx:
        consts = ctx.enter_context(tc.tile_pool(name="consts", bufs=1))
        imp = ctx.enter_context(tc.tile_pool(name="im", bufs=2))
        fpool = ctx.enter_context(tc.tile_pool(name="feats", bufs=3))
        opool = ctx.enter_context(tc.tile_pool(name="outs", bufs=3))
        pc = ctx.enter_context(tc.tile_pool(name="psc", bufs=2, space="PSUM"))
        plA = ctx.enter_context(tc.tile_pool(name="pslA", bufs=2, space="PSUM"))
        plB = ctx.enter_context(tc.tile_pool(name="pslB", bufs=2, space="PSUM"))

        wc_sb = consts.tile([KS, E], f16)
        nc.sync.dma_start(out=wc_sb[:, :], in_=wc_d[:, :])
        cb_sb = consts.tile([128, 1], f32)
        nc.sync.dma_start(out=cb_sb[:, :], in_=cb_d[:, :])
        w2_sb = consts.tile([128, P], f16)
        nc.sync.dma_start(out=w2_sb[:, :], in_=w2_d[:, :])
        b2_sb = consts.tile([128, 8 * P], f32)
        nc.sync.dma_start(out=b2_sb[:, :], in_=b2_d[:, :])

        n_super = (S + SUPER - 1) // SUPER
        for sc in range(n_super):
            sbase = sc * SUPER
            scount = min(SUPER, S - sbase)
            im = imp.tile([KS, SUPER], f16)
            # rows 0-2: xe shifted 0/1/2; rows 3-4: xo shifted 0/1 — both
            # contiguous in DRAM (overlapping row reads are fine).
            nc.sync.dma_start(
                out=im[0:3, 0:scount],
                in_=bass.AP(tensor=xe_d, offset=sbase, ap=[[1, 3], [1, scount]]),
            )
            nc.sync.dma_start(
                out=im[3:5, 0:scount],
                in_=bass.AP(tensor=xo_d, offset=sbase, ap=[[1, 2], [1, scount]]),
            )

            cbase = 0
            while cbase < scount:
                cn = min(CHUNK, scount - cbase)
                assert cn % 2 == 0
                nA = cn // 2
                j0 = cbase
                p0g = sbase + cbase  # global first position of this chunk
                full = nA == HALF

                # conv: two halves -> PSUM partitions 0-63 / 64-127.
                psc = pc.tile([128, HALF], f32)
                if full:
                    # permuted moving operand: psum col t*128 + r holds
                    # position j0 + 4r + t
                    rhsA = im[:, j0 : j0 + nA].rearrange("k (r t) -> k t r", t=4)
                    rhsB = im[:, j0 + nA : j0 + 2 * nA].rearrange(
                        "k (r t) -> k t r", t=4
                    )
                else:
                    rhsA = im[:, j0 : j0 + nA]
                    rhsB = im[:, j0 + nA : j0 + 2 * nA]
                nc.tensor.matmul(
                    out=psc[0:E, 0:nA], lhsT=wc_sb[:, :], rhs=rhsA,
                    start=True, stop=True,
                )
                nc.tensor.matmul(
                    out=psc[E : 2 * E, 0:nA], lhsT=wc_sb[:, :], rhs=rhsB,
                    start=True, stop=True,
                )

                feats = fpool.tile([128, HALF], f16)
                nc.scalar.activation(
                    out=feats[:, 0:nA], in_=psc[:, 0:nA], func=RELU,
                    bias=cb_sb[:, 0:1], scale=1.0,
                )

                # linear: position tiles of <=128 as stationary operands.
                m_tiles = [
                    (i * 128, min(128, nA - i * 128)) for i in range((nA + 127) // 128)
                ]
                mlen0 = m_tiles[0][1]
                assert all(ml == mlen0 for _, ml in m_tiles)
                nb = len(m_tiles)
                psA = plA.tile([128, HALF // 2], f32)
                psB = plB.tile([128, HALF // 2], f32)
                for bi, (mo, ml) in enumerate(m_tiles):
                    nc.tensor.matmul(
                        out=psA[0:ml, P * bi : P * bi + P],
                        lhsT=feats[0:E, mo : mo + ml],
                        rhs=w2_sb[0:E, :], start=True, stop=True,
                    )
                    nc.tensor.matmul(
                        out=psB[0:ml, P * bi : P * bi + P],
                        lhsT=feats[E : 2 * E, mo : mo + ml],
                        rhs=w2_sb[E : 2 * E, :], start=True, stop=True,
                    )

                ncols = nb * P
                outt = opool.tile([128, HALF], f32)
                nc.vector.tensor_add(
                    outt[0:mlen0, 0:ncols],
                    psA[0:mlen0, 0:ncols],
                    b2_sb[0:mlen0, 0:ncols],
                )
                nc.vector.tensor_add(
                    outt[0:mlen0, ncols : 2 * ncols],
                    psB[0:mlen0, 0:ncols],
                    b2_sb[0:mlen0, 0:ncols],
                )

                if full:
                    # s = p0g + h*512 + 4r + q ; sbuf col = h*256 + q*64 + p
                    dview = out_d[p0g : p0g + cn, :].rearrange(
                        "(h r q) p -> r h q p", h=2, q=4
                    )
                    sview = outt[:, :].rearrange("r (h q p) -> r h q p", h=2, q=4)
                else:
                    # s = p0g + h*nA + r ; sbuf col = h*64 + p
                    dview = out_d[p0g : p0g + cn, :].rearrange(
                        "(h r) p -> r h p", h=2
                    )
                    sview = outt[0:mlen0, 0 : 2 * ncols].rearrange(
                        "r (h p) -> r h p", h=2
                    )
                nc.sync.dma_start(out=dview, in_=sview)

                cbase += cn


def prep_shared(conv_w, conv_b, lin_w, lin_b):
    """Host-side prep of the (tiny, replicated) parameter tensors."""
    conv_w = np.asarray(conv_w, dtype=np.float32)
    conv_b = np.asarray(conv_b, dtype=np.float32)
    lin_w = np.asarray(lin_w, dtype=np.float32)
    lin_b = np.asarray(lin_b, dtype=np.float32)

    wk = conv_w[:, 0, :]  # [64, 5]
    wc = np.ascontiguousarray(wk[:, [0, 2, 4, 1, 3]].T).astype(np.float16)  # [5, 64]
    cb = np.ascontiguousarray(
        np.concatenate([conv_b, conv_b]).astype(np.float32)[:, None]
    )  # [128, 1]
    w2 = lin_w.T.astype(np.float16)  # [64e, 64p]
    w2s = np.ascontiguousarray(np.concatenate([w2, w2], axis=0))  # [128, 64]
    b2 = np.ascontiguousarray(
        np.tile(lin_b.astype(np.float32)[None, :], (128, 8))
    )  # [128, 512]
    return wc, cb, w2s, b2


def prep_inputs(audio_waveform, conv_w, conv_b, lin_w, lin_b):
    """Host-side shard + dtype/layout prep. Returns in_maps for the 8 cores."""
    x = np.asarray(audio_waveform, dtype=np.float32)
    assert x.shape == (B, T)
    xp = np.zeros((B, 2 * S_FULL + 4), dtype=np.float16)
    xp[:, 2 : 2 + T] = x.astype(np.float16)
    xe = xp[:, 0::2]  # [B, S_FULL + 2]
    xo = xp[:, 1::2]  # [B, S_FULL + 2]

    wc, cb, w2s, b2 = prep_shared(conv_w, conv_b, lin_w, lin_b)

    in_maps = []
    for c in range(N_CORES):
        b_i, h = divmod(c, 2)
        s0 = h * S_CORE
        in_maps.append(
            dict(
                xe=np.ascontiguousarray(xe[b_i, s0 : s0 + S_CORE + 2]),
                xo=np.ascontiguousarray(xo[b_i, s0 : s0 + S_CORE + 2]),
                wc=wc, cb=cb, w2=w2s, b2=b2,
            )
        )
    return in_maps


_NC_CACHE = None


def get_nc() -> bass.Bass:
    global _NC_CACHE
    if _NC_CACHE is None:
        nc = bacc.Bacc()
        emit(nc)
        # Legalizes TRN2 sync constraints (splits multi-wait instructions),
        # allocates registers, etc. Required before walrus codegen.
        nc.compile()
        _NC_CACHE = nc
    return _NC_CACHE


def run(inputs: dict, trace: bool = False):
    """Run on the 8 cores; returns (full_output, BassKernelResults)."""
    in_maps = prep_inputs(**inputs)
    nc = get_nc()
    res = run_bass_kernel_spmd(nc, in_maps, list(range(N_CORES)), trace=trace)
    out = np.empty((B, S_FULL, P), dtype=np.float32)
    for c in range(N_CORES):
        b_i, h = divmod(c, 2)
        out[b_i, h * S_CORE : (h + 1) * S_CORE, :] = res.results[c]["out"]
    return out, res


def kernel(**inputs) -> np.ndarray:
    out, _ = run(inputs)
    return out



# revision 4
# speedup vs baseline: 1.1882x; 1.1882x over previous
"""AudioEncoder Trainium2 kernel.

Computes: conv1d(1->64, k=5, stride=2, pad=2) + bias -> ReLU -> per-timestep
linear (64->64) + bias, over audio [4, 480000] f32 -> out [4, 240000, 64] f32.

Strategy (pure data parallel over 8 cores):
  - Each core handles one half of one batch row: S = 120000 output positions.
  - Host pre-pads/casts audio to fp16 and de-interleaves it into even/odd
    streams xe[i] = xp[2i], xo[i] = xp[2i+1] (xp[t] = x[t-2] zero-padded), so
    the on-chip im2col rows are contiguous DMA reads:
      row0 = xe[j]   (tap 0)   row3 = xo[j]   (tap 1)
      row1 = xe[1+j] (tap 2)   row4 = xo[1+j] (tap 3)
      row2 = xe[2+j] (tap 4)
    (conv weights are host-reordered to [k=0,2,4,1,3] to match).  The im2col
    rows are loaded twice, at partitions 0-4 and 32-36, so the conv matmuls
    can use two PE row groups (and the loads use two SBUF ports).
  - Chunk = 2048 positions.  Conv: four CONCURRENT k=5 fp16 matmuls on
    disjoint PE quadrants (row groups 0/32 x col groups 0/64) fill one
    [128, 1024] PSUM tile (2 banks): partitions 0-63 hold channels for
    positions [b, b+1024), partitions 64-127 for [b+1024, b+2048).  The
    moving operand uses a permuted 3D AP so psum/feats column c = bi*128 + r
    holds position (half base) + 8*r + bi  (bi = c//128 in 0..7).
  - ACT applies conv bias + ReLU in ONE [128, 1024] op (PSUM -> SBUF fp16).
  - Linear: 16 fp16 matmuls per chunk; feats 128-col blocks stationary,
    lin_w.T moving.  A (feats rows 0-63) and B (rows 64-127) matmuls run
    concurrently in different PE row groups and write the two separate PSUM
    banks of one [128, 1024] tile (same-partition same-bank concurrent
    writes are a HW fault; different banks are safe).
  - DVE adds the (pre-broadcast) linear bias in ONE [128, 1024] op.
  - The 8-position interleave means each SBUF partition holds 8+8 consecutive
    output rows, so the store DMA moves 2 KiB-contiguous runs (512 KiB/DMA).
  - Input loads go on the GPSIMD (SWDGE) queue so their packets do not
    FIFO-block the store packets on the sync HWDGE queue.
  - PSUM budget: conv 2x[128,1024] + linear 2x[128,1024] double-buffered
    = exactly 8 banks.
"""

import numpy as np

import concourse.bacc as bacc
import concourse.bass as bass
import concourse.mybir as mybir
import concourse.tile as tile
from concourse.bass_utils import run_bass_kernel_spmd

B = 4
T = 480000
S_FULL = 240000  # conv output positions per batch row
N_CORES = 8
S_CORE = S_FULL * B // N_CORES  # 120000 positions per core
CHUNK = 2048  # output positions per chunk
SUPER = 8192  # output positions covered per im2col load
E = 64  # conv out channels
P = 64  # linear out features
KS = 5

f16 = mybir.dt.float16
f32 = mybir.dt.float32


def emit(nc: bass.Bass, S: int = S_CORE) -> None:
    """Emit the per-core Tile kernel for S output positions."""
    from contextlib import ExitStack

    xe_d = nc.declare_dram_parameter("xe", [S + 2], f16, isOutput=False)
    xo_d = nc.declare_dram_parameter("xo", [S + 2], f16, isOutput=False)
    wc_d = nc.declare_dram_parameter("wc", [40, E], f16, isOutput=False)
    cb_d = nc.declare_dram_parameter("cb", [128, 1], f32, isOutput=False)
    w2_d = nc.declare_dram_parameter("w2", [128, P], f16, isOutput=False)
    b2_d = nc.declare_dram_parameter("b2", [128, 16 * P], f32, isOutput=False)
    out_d = nc.declare_dram_parameter("out", [S, P], f32, isOutput=True)

    RELU = mybir.ActivationFunctionType.Relu
    HALF = CHUNK // 2  # 1024

    # chunk bases: full 2048-chunks; the ragged tail is covered by one final
    # chunk at S-CHUNK that overlap-recomputes (and harmlessly rewrites) up
    # to 2047 already-stored positions.
    bases = list(range(0, S - CHUNK + 1, CHUNK))
    if bases[-1] + CHUNK < S:
        bases.append(S - CHUNK)

    with tile.TileContext(nc) as tc, ExitStack() as ctx:
        consts = ctx.enter_context(tc.tile_pool(name="consts", bufs=1))
        imp = ctx.enter_context(tc.tile_pool(name="im", bufs=2))
        fpool = ctx.enter_context(tc.tile_pool(name="feats", bufs=3))
        opool = ctx.enter_context(tc.tile_pool(name="outs", bufs=3))
        pc = ctx.enter_context(tc.tile_pool(name="psc", bufs=2, space="PSUM"))
        pl = ctx.enter_context(tc.tile_pool(name="psl", bufs=2, space="PSUM"))

        wc_sb = consts.tile([40, E], f16)
        nc.sync.dma_start(out=wc_sb[:, :], in_=wc_d[:, :])
        cb_sb = consts.tile([128, 1], f32)
        nc.sync.dma_start(out=cb_sb[:, :], in_=cb_d[:, :])
        w2_sb = consts.tile([128, P], f16)
        nc.sync.dma_start(out=w2_sb[:, :], in_=w2_d[:, :])
        b2_sb = consts.tile([128, 16 * P], f32)
        nc.sync.dma_start(out=b2_sb[:, :], in_=b2_d[:, :])

        im = None
        cur_super = -1
        for b in bases:
            sbase = b // SUPER * SUPER
            if sbase != cur_super:
                cur_super = sbase
                scount = min(SUPER, S - sbase)
                im = imp.tile([37, SUPER], f16)
                # rows 0-2: xe shifted 0/1/2; rows 3-4: xo shifted 0/1 — both
                # contiguous in DRAM (overlapping row reads are fine).  Loaded
                # twice: partitions 0-4 (row group 0) and 32-36 (row group 1).
                for pbase in (0, 32):
                    nc.gpsimd.dma_start(
                        out=im[pbase : pbase + 3, 0:scount],
                        in_=bass.AP(
                            tensor=xe_d, offset=sbase, ap=[[1, 3], [1, scount]]
                        ),
                    )
                    nc.gpsimd.dma_start(
                        out=im[pbase + 3 : pbase + 5, 0:scount],
                        in_=bass.AP(
                            tensor=xo_d, offset=sbase, ap=[[1, 2], [1, scount]]
                        ),
                    )

            c0 = b - sbase
            # conv: 4 concurrent matmuls on disjoint PE quadrants.
            psc = pc.tile([128, HALF], f32)  # 2 banks
            for h in (0, 1):  # psum partition half <-> position half
                v0 = im[0:KS, c0 + HALF * h : c0 + HALF * h + HALF].rearrange(
                    "k (r t) -> k t r", t=8
                )
                v1 = im[32 : 32 + KS, c0 + HALF * h : c0 + HALF * h + HALF].rearrange(
                    "k (r t) -> k t r", t=8
                )
                nc.tensor.matmul(
                    out=psc[E * h : E * h + E, 0:512],
                    lhsT=wc_sb[0:KS, :],
                    rhs=v0[:, 0:4, :],
                    start=True, stop=True,
                )
                nc.tensor.matmul(
                    out=psc[E * h : E * h + E, 512:1024],
                    lhsT=wc_sb[32 : 32 + KS, :],
                    rhs=v1[:, 4:8, :],
                    start=True, stop=True,
                )

            feats = fpool.tile([128, HALF], f16)
            nc.scalar.activation(
                out=feats[:, :], in_=psc[:, :], func=RELU,
                bias=cb_sb[:, 0:1], scale=1.0,
            )

            # linear: 8 A/B concurrent pairs; A -> bank 0, B -> bank 1.
            psl = pl.tile([128, HALF], f32)  # 2 banks
            for bi in range(8):
                nc.tensor.matmul(
                    out=psl[:, P * bi : P * bi + P],
                    lhsT=feats[0:E, 128 * bi : 128 * bi + 128],
                    rhs=w2_sb[0:E, :], start=True, stop=True,
                )
                nc.tensor.matmul(
                    out=psl[:, 512 + P * bi : 512 + P * bi + P],
                    lhsT=feats[E : 2 * E, 128 * bi : 128 * bi + 128],
                    rhs=w2_sb[E : 2 * E, :], start=True, stop=True,
                )

            outt = opool.tile([128, HALF], f32)
            nc.vector.tensor_add(outt[:, :], psl[:, :], b2_sb[:, :])

            # s = b + h*1024 + 8r + t ; sbuf col = h*512 + t*64 + p
            dview = out_d[b : b + CHUNK, :].rearrange(
                "(h r t) p -> r h t p", h=2, t=8
            )
            sview = outt[:, :].rearrange("r (h t p) -> r h t p", h=2, t=8)
            nc.sync.dma_start(out=dview, in_=sview)


def prep_shared(conv_w, conv_b, lin_w, lin_b):
    """Host-side prep of the (tiny, replicated) parameter tensors."""
    conv_w = np.asarray(conv_w, dtype=np.float32)
    conv_b = np.asarray(conv_b, dtype=np.float32)
    lin_w = np.asarray(lin_w, dtype=np.float32)
    lin_b = np.asarray(lin_b, dtype=np.float32)

    wk = conv_w[:, 0, :]  # [64, 5]
    wc5 = wk[:, [0, 2, 4, 1, 3]].T.astype(np.float16)  # [5, 64]
    wc = np.zeros((40, E), dtype=np.float16)
    wc[0:5] = wc5
    wc[32:37] = wc5
    cb = np.ascontiguousarray(
        np.concatenate([conv_b, conv_b]).astype(np.float32)[:, None]
    )  # [128, 1]
    w2 = lin_w.T.astype(np.float16)  # [64e, 64p]
    w2s = np.ascontiguousarray(np.concatenate([w2, w2], axis=0))  # [128, 64]
    b2 = np.ascontiguousarray(
        np.tile(lin_b.astype(np.float32)[None, :], (128, 16))
    )  # [128, 1024]
    return wc, cb, w2s, b2


def prep_inputs(audio_waveform, conv_w, conv_b, lin_w, lin_b):
    """Host-side shard + dtype/layout prep. Returns in_maps for the 8 cores."""
    x = np.asarray(audio_waveform, dtype=np.float32)
    assert x.shape == (B, T)
    xp = np.zeros((B, 2 * S_FULL + 4), dtype=np.float16)
    xp[:, 2 : 2 + T] = x.astype(np.float16)
    xe = xp[:, 0::2]  # [B, S_FULL + 2]
    xo = xp[:, 1::2]  # [B, S_FULL + 2]

    wc, cb, w2s, b2 = prep_shared(conv_w, conv_b, lin_w, lin_b)

    in_maps = []
    for c in range(N_CORES):
        b_i, h = divmod(c, 2)
        s0 = h * S_CORE
        in_maps.append(
            dict(
                xe=np.ascontiguousarray(xe[b_i, s0 : s0 + S_CORE + 2]),
                xo=np.ascontiguousarray(xo[b_i, s0 : s0 + S_CORE + 2]),
                wc=wc, cb=cb, w2=w2s, b2=b2,
            )
        )
    return in_maps


_NC_CACHE = None


def get_nc() -> bass.Bass:
    global _NC_CACHE
    if _NC_CACHE is None:
        nc = bacc.Bacc()
        emit(nc)
        # Legalizes TRN2 sync constraints (splits multi-wait instructions),
        # allocates registers, etc. Required before walrus codegen.
        nc.compile()
        _NC_CACHE = nc
    return _NC_CACHE


def run(inputs: dict, trace: bool = False):
    """Run on the 8 cores; returns (full_output, BassKernelResults)."""
    in_maps = prep_inputs(**inputs)
    nc = get_nc()
    res = run_bass_kernel_spmd(nc, in_maps, list(range(N_CORES)), trace=trace)
    out = np.empty((B, S_FULL, P), dtype=np.float32)
    for c in range(N_CORES):
        b_i, h = divmod(c, 2)
        out[b_i, h * S_CORE : (h + 1) * S_CORE, :] = res.results[c]["out"]
    return out, res


def kernel(**inputs) -> np.ndarray:
    out, _ = run(inputs)
    return out
